# revision 35
# baseline (speedup 1.0000x reference)
"""Multi-head causal attention (B=2, S=2048, D=1024, H=16) on 8 TRN2 NeuronCores.

Sharding: batch*head parallel. Core c handles batch b = c//4 and the 4
heads h in [4*(c%4), 4*(c%4)+4). Each core computes its heads' Q/K/V
projections (column-parallel), causal softmax attention, and its partial
row-parallel output projection; the host sums the 4 partial outputs per
batch (the AllReduce of row-parallel tensor parallelism).

Streamed-chunk schedule: x is brought in as 512-column chunks (q/k/v per
round) and projected in PE bursts between attention blocks, so the
DMA-bound projection phase hides entirely under the PE/exp-bound
attention stream and the PE never idles long enough to drop the HAM
clock gate to half speed. Per round c: attention(pr0,c) -> output
projection of block c-1 -> attention(pr1,c) -> projection of chunk c+1
-> DMA issue for chunk c+2.

On-device layout: everything is kept "transposed" (feature-major) so
every matmul contracts along the partition dimension:
  scoresT[k,q] = K Q^T      (per head, 128-row k-tiles x 512-col q-tiles)
  P^T = exp(scoresT/8 + mask/8)   (additive -1e9 causal mask, PE-accumulated)
  outT[d,q]   = sum_k V[k,d] P^T[k,q]   (PSUM-accumulated over k-tiles)
  sums[q]     = sum_k P^T[k,q]          (ones-vector matmul, col-packed)
  y[q,e]     += sum_hd outT_norm[hd,q] * w_oT[hd,e]
Softmax skips the max-subtraction: scores ~ N(0,1), so exp never
overflows fp32, and exp(-1e9/8) underflows to exactly 0 like the
reference's masked_fill(-1e9).

Projections run as float32r (TF32-like); scores/attnV/output-projection
run bf16 (measured ~30% faster per moving row on this part). The softmax
normalize runs entirely off the PE: reciprocal on DVE, partition
broadcast on GPSIMD, so the in-order PE queue never stalls on it.
Fully-masked 128x512 blocks are skipped (causal => ~62% computed).
"""

import numpy as np

D_MODEL = 1024
N_HEADS = 16
D_K = 64
B, S = 2, 2048
N_CORES = 8
HPC = 4              # heads per core
KT = S // 128        # 16 k-tiles
QT = S // 512        # 4 q-tiles == x chunks
ET = D_MODEL // 128  # 8 e-tiles (contraction tiles for projections)

WARM0 = 16           # initial PE warm-up matmuls (cover the first DMAs)
WARM_TAIL = 15        # PE filler while the last normalize chain runs

_PROG_CACHE = {}


def _build_program():
    import concourse.bacc as bacc_mod
    import concourse.mybir as mybir
    import concourse.tile as tile

    f32 = mybir.dt.float32
    f32r = mybir.dt.float32r
    bf16 = mybir.dt.bfloat16
    Exp = mybir.ActivationFunctionType.Exp
    Copy = mybir.ActivationFunctionType.Copy

    nc = bacc_mod.Bacc(
        "TRN2", target_bir_lowering=False, debug=False, num_devices=N_CORES
    )

    # x tensors are host-permuted per 512-col chunk: row p*8+t of chunk c is
    # x[t*128+p, c*512:(c+1)*512], so each chunk DMA is a contiguous 2D
    # transfer (16KB per partition). Same for weights (8KB per partition).
    xq = nc.dram_tensor("xq", [QT * D_MODEL, 512], bf16, kind="ExternalInput").ap()
    xk = nc.dram_tensor("xk", [QT * D_MODEL, 512], bf16, kind="ExternalInput").ap()
    xv = nc.dram_tensor("xv", [QT * D_MODEL, 512], bf16, kind="ExternalInput").ap()
    wq = nc.dram_tensor("wq", [D_MODEL, 256], bf16, kind="ExternalInput").ap()
    wk = nc.dram_tensor("wk", [D_MODEL, 256], bf16, kind="ExternalInput").ap()
    wv = nc.dram_tensor("wv", [D_MODEL, 256], bf16, kind="ExternalInput").ap()
    wo = nc.dram_tensor("wo", [256, D_MODEL], bf16, kind="ExternalInput").ap()
    maskt = nc.dram_tensor("maskt", [128, 2048], bf16, kind="ExternalInput").ap()
    idbf = nc.dram_tensor("idbf", [128, 132], bf16, kind="ExternalInput").ap()
    y = nc.dram_tensor("y", [S, D_MODEL], bf16, kind="ExternalOutput").ap()

    with (
        tile.TileContext(nc) as tc,
        nc.allow_low_precision("bf16/fp32r attention"),
        tc.tile_pool(name="persist", bufs=1) as pp,
        tc.tile_pool(name="xc", bufs=3) as xcp,
        tc.tile_pool(name="aux", bufs=2) as auxp,
        tc.tile_pool(name="psS", bufs=2, space="PSUM") as psS,
        tc.tile_pool(name="psO", bufs=2, space="PSUM") as psO,
    ):
        etp = rcp = bcp = ysbp = auxp
        # ---- persistent SBUF tiles ----
        def persist(shape, dtype, name):
            return pp.tile(shape, dtype, name=name, tag=name)

        wq_sb = persist([128, ET * 256], bf16, "wq_sb")
        wk_sb = persist([128, ET * 256], bf16, "wk_sb")
        wv_sb = persist([128, ET * 256], bf16, "wv_sb")
        wo_sb = [persist([128, D_MODEL], bf16, f"wo_sb{p}") for p in range(2)]
        maskt_sb = persist([128, 2048], bf16, "maskt_sb")
        idbf_sb = persist([128, 132], bf16, "idbf_sb")
        # projected Q for the current round, double-buffered by round parity
        qt_blk = [
            [persist([128, 512], bf16, f"qt_blk{par}_{m}") for m in range(2)]
            for par in range(2)
        ]
        kt_sb = [persist([128, S], bf16, f"kt_sb{m}") for m in range(2)]
        v_sb = [persist([128, 260], bf16, f"v_sb{i}") for i in range(KT)]
        outt_sb = [persist([128, S], bf16, f"outt_sb{m}") for m in range(2)]

        # ---- DMA emitters (sync queue; emission order == issue order) ----
        # weights and x are host-permuted so both loads are contiguous 2D
        # transfers (row p*8+t holds e-tile t's partition-p slice).
        def emit_w_dma(w_dram, w_tile):
            # weight load: [(p t), 256] -> [128, 8*256] (e-tile t at cols 256t)
            nc.sync.dma_start(
                out=w_tile[:],
                in_=w_dram.rearrange("(p t) d -> p (t d)", p=128),
            )

        def emit_x_chunk(x_dram, c, nm, eng=None):
            # x chunk c: [(p t), 512] -> [128, 8*512] (e-tile t at cols 512t)
            t = xcp.tile([128, ET * 512], bf16, name=f"{nm}{c}", tag="xc")
            (eng or nc.sync).dma_start(
                out=t[:],
                in_=x_dram[c * D_MODEL : (c + 1) * D_MODEL, :].rearrange(
                    "(p t) k -> p (t k)", p=128
                ),
            )
            return t

        # ---- PE warm-up filler ----
        # The HAM clock gate drops the PE to half clock after any multi-us
        # idle and needs ~3.4us of gapless activity to recover; dummy
        # matmuls (results never read) bridge unavoidable DMA-bound waits.
        def emit_warm(n, name):
            wt = psS.tile([128, 1024], f32, name=name, tag="s")
            for _ in range(n):
                nc.tensor.matmul(
                    wt[:, 0:512], idbf_sb[:, 0:128], maskt_sb[:, 0:512],
                    start=True, stop=True,
                )

        # ---- projection bursts ----
        def emit_proj_qk(c, xt, w_tile, is_q, warm_n=0):
            nm = "q" if is_q else "k"
            ps = psS.tile([128, 1024], f32, name=f"psp{nm}_{c}", tag="s")
            # warm-up filler matmuls into the live tile (the real e==0 start
            # clears them); a standalone never-read warm tile gets optimized
            # away and leaves the PE idle
            for _ in range(warm_n):
                nc.tensor.matmul(
                    ps[:, 0:512], idbf_sb[:, 0:128], maskt_sb[:, 0:512],
                    start=True, stop=True,
                )
            for e in range(ET):
                for m in range(2):
                    nc.tensor.matmul(
                        ps[:, m * 512 : (m + 1) * 512],
                        w_tile[:, e * 256 + m * 128 : e * 256 + (m + 1) * 128],
                        xt[:, e * 512 : (e + 1) * 512],
                        start=(e == 0),
                        stop=(e == ET - 1),
                    )
            for m in range(2):
                dst = (
                    qt_blk[c % 2][m][:]
                    if is_q
                    else kt_sb[m][:, c * 512 : (c + 1) * 512]
                )
                nc.vector.tensor_copy(dst, ps[:, m * 512 : (m + 1) * 512])

        def emit_proj_v(c, xt):
            # one PSUM accumulation stream per bank: the matmul start flag
            # clears the whole bank, so two independent 256-wide streams must
            # not share one. Each [128,1024] tile hosts 2 k-tiles at bank
            # starts (cols 0 and 512).
            for half in range(2):
                ps = psS.tile([128, 1024], f32, name=f"pspv_{c}_{half}", tag="s")
                for e in range(ET):
                    for kk in range(2):
                        ktl = 2 * half + kk
                        nc.tensor.matmul(
                            ps[:, kk * 512 : kk * 512 + 256],
                            xt[:, e * 512 + ktl * 128 : e * 512 + (ktl + 1) * 128],
                            wv_sb[:, e * 256 : (e + 1) * 256],
                            start=(e == 0),
                            stop=(e == ET - 1),
                        )
                for kk in range(2):
                    ktl = 2 * half + kk
                    i = 4 * c + ktl
                    nc.vector.tensor_copy(
                        v_sb[i][:].rearrange("p (h c) -> p h c", c=65)[:, :, 0:64],
                        ps[:, kk * 512 : kk * 512 + 256].rearrange(
                            "p (h d) -> p h d", d=64
                        ),
                    )
                    nc.vector.tensor_copy(
                        v_sb[i][:].rearrange("p (h c) -> p h c", c=65)[:, :, 64:65],
                        idbf_sb[:, 128:132].rearrange("p (h c) -> p h c", c=1),
                    )

        # ---- output projection (one 128-row m-tile of y) ----
        def emit_outproj_mtile(m, warm_n=0):
            psy = psS.tile([128, 1024], f32, name=f"psy_{m}", tag="s")
            for _ in range(warm_n):
                nc.tensor.matmul(
                    psy[:, 0:512], idbf_sb[:, 0:128], maskt_sb[:, 0:512],
                    start=True, stop=True,
                )
            for p in range(2):
                for n in range(2):
                    nc.tensor.matmul(
                        psy[:, n * 512 : (n + 1) * 512],
                        outt_sb[p][:, m * 128 : (m + 1) * 128],
                        wo_sb[p][:, n * 512 : (n + 1) * 512],
                        start=(p == 0),
                        stop=(p == 1),
                    )
            y_sb = ysbp.tile([128, 1024], bf16, name=f"y_sb_{m}", tag="ysb", bufs=4)
            # scalar engine stages y out of PSUM: it idles during bursts and
            # this keeps the DVE free for the normalize chain
            nc.scalar.activation(y_sb[:], psy[:], Copy)
            nc.sync.dma_start(out=y[m * 128 : (m + 1) * 128, :], in_=y_sb[:])

        # ---- softmax normalize: no PE involvement ----
        def emit_normalize(pr, jj, ps_out_prev):
            qsj = slice(jj * 512, (jj + 1) * 512)
            ssb = rcp.tile([1, 1024], f32, name=f"ssb_{pr}_{jj}", tag="ssb")
            nc.vector.tensor_copy(ssb[0:1, :], ps_out_prev[64:65, :])
            rc = rcp.tile([1, 1024], f32, name=f"rc_{pr}_{jj}", tag="rc")
            nc.vector.reciprocal_approx_fast(out=rc[:], in_=ssb[:])
            bc_sb = bcp.tile([64, 1024], f32, name=f"bc_sb_{pr}_{jj}", tag="bc")
            nc.gpsimd.partition_broadcast(bc_sb[:], rc[0:1, :])
            for hh in range(2):
                nc.vector.tensor_mul(
                    outt_sb[pr][64 * hh : 64 * hh + 64, qsj],
                    ps_out_prev[0:64, 512 * hh : 512 * (hh + 1)],
                    bc_sb[:, 512 * hh : 512 * (hh + 1)],
                )

        # ---- startup: consts + weights + chunk 0, projection 0 ----
        # warm-up source tile seeded by memset before any DMA is issued: the
        # PE ramp starts as soon as the queues open
        wz = pp.tile([128, 512], bf16, name="wz", tag="wz")
        nc.gpsimd.memset(wz[:], 0.0)
        wzp = psS.tile([128, 1024], f32, name="wz_ps", tag="s")
        for _ in range(WARM0):
            nc.tensor.matmul(
                wzp[:, 0:512], wz[:, 0:128], wz[:, 0:512],
                start=True, stop=True,
            )
        # weights stream on the sync queue while the first x chunks stream on
        # the (otherwise idle) gpsimd queue — parallel issue and transfer.
        nc.gpsimd.dma_start(out=idbf_sb[:], in_=idbf[:])
        nc.sync.dma_start(out=maskt_sb[:], in_=maskt[:])
        emit_w_dma(wq, wq_sb)
        xq_t = emit_x_chunk(xq, 0, "xq", nc.gpsimd)
        emit_w_dma(wk, wk_sb)
        xk_t = emit_x_chunk(xk, 0, "xk", nc.scalar)
        emit_w_dma(wv, wv_sb)
        xv_t = emit_x_chunk(xv, 0, "xv", nc.gpsimd)
        for p in range(2):
            nc.sync.dma_start(out=wo_sb[p][:], in_=wo[p * 128 : (p + 1) * 128, :])

        emit_proj_qk(0, xq_t, wq_sb, True, warm_n=4)
        emit_proj_qk(0, xk_t, wk_sb, False)
        emit_proj_v(0, xv_t)
        nxt = (
            emit_x_chunk(xq, 1, "xq"),
            emit_x_chunk(xk, 1, "xk"),
            emit_x_chunk(xv, 1, "xv"),
        )

        # ---- rounds ----
        pending_norm = None  # (pr, j, ps_out) awaiting lazy normalize
        pending_out = []     # m-tiles awaiting output projection

        def emit_attention_block(pr, j):
            nonlocal pending_norm
            n_i = 4 * j + 4
            ps_out = psO.tile([65, 1024], f32, name=f"ps_out_{pr}_{j}", tag="o")
            prev_et = None
            prev_i = -1
            for i in range(n_i):
                diag = i >= 4 * j
                r = i - 4 * j
                pss = psS.tile([128, 1024], f32, name=f"ps_s{pr}_{j}_{i}", tag="s")
                if diag:
                    nw = 128 * (r + 1)
                    for hh in range(2):
                        nc.tensor.matmul(
                            pss[:, 512 * hh : 512 * hh + nw],
                            idbf_sb[:, 0:128],
                            maskt_sb[:, r * 512 : r * 512 + nw],
                            start=True,
                            stop=False,
                        )
                for hh in range(2):
                    hp = slice(64 * hh, 64 * hh + 64)
                    nc.tensor.matmul(
                        pss[:, 512 * hh : 512 * (hh + 1)],
                        kt_sb[pr][hp, i * 128 : (i + 1) * 128],
                        qt_blk[j % 2][pr][hp, :],
                        start=not diag,
                        stop=True,
                    )
                et = etp.tile([128, 1024], bf16, name=f"et{pr}_{j}_{i}", tag="et", bufs=6)
                nc.scalar.activation(et[:], pss[:], Exp, scale=0.125)
                if prev_et is not None:
                    for hh in range(2):
                        nc.tensor.matmul(
                            ps_out[:, 512 * hh : 512 * (hh + 1)],
                            v_sb[prev_i][:, (2 * pr + hh) * 65 : (2 * pr + hh + 1) * 65],
                            prev_et[:, 512 * hh : 512 * (hh + 1)],
                            start=(prev_i == 0),
                            stop=(prev_i == n_i - 1),
                        )
                prev_et, prev_i = et, i
                if i == 1 and pending_norm is not None:
                    pn_pr, pn_j = pending_norm[0], pending_norm[1]
                    emit_normalize(*pending_norm)
                    pending_norm = None
                    if pn_pr == 1:
                        pending_out.extend(range(4 * pn_j, 4 * pn_j + 4))
            for hh in range(2):
                nc.tensor.matmul(
                    ps_out[:, 512 * hh : 512 * (hh + 1)],
                    v_sb[n_i - 1][:, (2 * pr + hh) * 65 : (2 * pr + hh + 1) * 65],
                    prev_et[:, 512 * hh : 512 * (hh + 1)],
                    start=(n_i - 1 == 0),
                    stop=True,
                )
            pending_norm = (pr, j, ps_out)

        for c in range(QT):
            emit_attention_block(0, c)
            # outproj of block c-1 (normalize(pr1,c-1) fired at (pr0,c) i==1);
            # warm filler on the first tile bridges that normalize's DVE/GPSIMD
            # chain. In the last round, half the batch is kept back to fill
            # the PE during the final block's normalize at the tail.
            first = True
            while pending_out and c != QT - 1:
                emit_outproj_mtile(pending_out.pop(0), warm_n=2 if first else 0)
                first = False
            emit_attention_block(1, c)
            if c + 1 < QT:
                emit_proj_qk(c + 1, nxt[0], wq_sb, True)
                emit_proj_qk(c + 1, nxt[1], wk_sb, False)
                emit_proj_v(c + 1, nxt[2])
                if c + 2 < QT:
                    nxt = (
                        emit_x_chunk(xq, c + 2, "xq"),
                        emit_x_chunk(xk, c + 2, "xk"),
                        emit_x_chunk(xv, c + 2, "xv"),
                    )

        # ---- tail: last block's normalize + output projection ----
        # the held-back outproj tiles of block QT-2 are real PE work that
        # covers the final normalize chain's latency
        emit_normalize(*pending_norm)
        while pending_out:
            emit_outproj_mtile(pending_out.pop(0))
        for m in range(4 * (QT - 1), 4 * QT):
            emit_outproj_mtile(m)

    nc.compile()
    return nc


def _get_program():
    if "nc" not in _PROG_CACHE:
        _PROG_CACHE["nc"] = _build_program()
    return _PROG_CACHE["nc"]


def _host_prep(query, key, value, mask, w_q, w_k, w_v, w_o):
    import ml_dtypes

    query = np.asarray(query, dtype=np.float32)
    key = np.asarray(key, dtype=np.float32)
    value = np.asarray(value, dtype=np.float32)
    w_q = np.asarray(w_q, dtype=np.float32)
    w_k = np.asarray(w_k, dtype=np.float32)
    w_v = np.asarray(w_v, dtype=np.float32)
    w_o = np.asarray(w_o, dtype=np.float32)
    m = np.asarray(mask).reshape(S, S).astype(bool)

    # The kernel's block-skip structure assumes the standard causal mask.
    expected = np.triu(np.ones((S, S), dtype=bool), k=1)
    if not np.array_equal(m, expected):
        raise NotImplementedError("kernel specialized for causal (triu, k=1) mask")

    # 4 canonical diagonal-straddle mask tiles: pattern r covers k-tile
    # 4j+r vs q-tile j; masked where (128r + row) > col.
    maskt = np.zeros((128, 2048), dtype=np.float32)
    rows = np.arange(128)[:, None]
    cols = np.arange(512)[None, :]
    for r in range(4):
        maskt[:, r * 512 : (r + 1) * 512] = np.where(
            (128 * r + rows) > cols, np.float32(-1e9), np.float32(0.0)
        )
    maskt = maskt.astype(ml_dtypes.bfloat16)
    idbf = np.zeros((128, 132), dtype=ml_dtypes.bfloat16)
    idbf[:, 0:128] = np.eye(128, dtype=ml_dtypes.bfloat16)
    idbf[:, 128:132] = ml_dtypes.bfloat16(1.0)

    def permute_x(x):
        # x[S, D] -> chunks[(c p t), k] with row p*8+t of chunk c holding
        # x.T[t*128+p, c*512:(c+1)*512] (contiguous per-partition DMA)
        xt_ = x.T.reshape(ET, 128, QT, 512)          # [t, p, c, k]
        return np.ascontiguousarray(
            xt_.transpose(2, 1, 0, 3).reshape(QT * D_MODEL, 512)
        ).astype(ml_dtypes.bfloat16)

    def permute_w(w_rows):
        # w[256 out, D in] -> [(p t), 256] with row p*8+t = w.T[t*128+p, :]
        wt_ = w_rows.T.reshape(ET, 128, 256)         # [t, p, d]
        return np.ascontiguousarray(
            wt_.transpose(1, 0, 2).reshape(D_MODEL, 256)
        ).astype(ml_dtypes.bfloat16)

    xt = {}
    for b in range(B):
        xt[("q", b)] = permute_x(query[b])
        xt[("k", b)] = permute_x(key[b])
        xt[("v", b)] = permute_x(value[b])

    in_maps = []
    for c in range(N_CORES):
        b = c // 4
        hb = (c % 4) * HPC
        rs = slice(hb * D_K, (hb + HPC) * D_K)
        in_maps.append(
            {
                "xq": xt[("q", b)],
                "xk": xt[("k", b)],
                "xv": xt[("v", b)],
                "wq": permute_w(w_q[rs, :]),
                "wk": permute_w(w_k[rs, :]),
                "wv": permute_w(w_v[rs, :]),
                "wo": np.ascontiguousarray(w_o[:, rs].T).astype(ml_dtypes.bfloat16),
                "maskt": maskt,
                "idbf": idbf,
            }
        )
    return in_maps


def kernel(query, key, value, mask, w_q, w_k, w_v, w_o):
    from concourse.bass_utils import run_bass_kernel_spmd

    in_maps = _host_prep(query, key, value, mask, w_q, w_k, w_v, w_o)
    nc = _get_program()
    res = run_bass_kernel_spmd(nc, in_maps, list(range(N_CORES)))
    out = np.zeros((B, S, D_MODEL), dtype=np.float32)
    for c in range(N_CORES):
        out[c // 4] += res.results[c]["y"].astype(np.float32)
    return out
